# revision 22
# baseline (speedup 1.0000x reference)
"""Trainium2 Bass kernel for HadamardTernaryLinear.

y = reshape( (FHT_g(x*alpha) @grouped w_q) -> FHT_h -> *beta ), with
w_q = BitNet-style absmean ternary quantization of weight.

Strategy: data-parallel over the 8192 tokens across 8 NeuronCores (1024
tokens/core, no collectives). Per core, a 5-pass TensorEngine pipeline in
fp16 (Hadamard and ternary weight matrices are exact +-1/0 in fp16; the
quant scale and alpha are folded host-side; beta*scale/32 is folded into
the per-m P5 stationaries):

  P1 stat-FHTg : MM(lhsT=x-tile, rhs=I4(x)H32)   -> token-major  [tok, (i,h)]
  P2 T-regroup : MM(lhsT=sel,    rhs=I)          -> group-major  [i, tok] per h
  P3 stat-GM   : MM(lhsT=xb,     rhs=wqT[h])     -> token-major  [tok, (h,o)]
  P4 T-regroup : MM(lhsT=sel,    rhs=I)          -> A-layout     [(o',h), tok]
  P5 mov-FHTh  : MM(lhsT=beta_m*I4(x)H32, rhs=ypa) -> [(o',g), tok]
  plain-copy drains throughout; PSUM drained in [128,1024] batches split
  across DVE + Act. Optionally (TRANSPOSE_MODE) P2/P4 run in PE transpose
  mode writing fp16 PSUM so their drains hit the DVE 2x path.

x is pre-transposed host-side to feature-major per core (feature order
i-major, f' = i*32+g) and pre-tiled per supertile so every DMA moves
fully contiguous per-partition runs (the device DMA-xbar transpose of
2-byte elements is catastrophically slow and must never be used).
Loads ride the SP HW-DGE queue, stores the Act queue. Output leaves the
device fp16, supertile-tiled; the host unpacks.
"""

import functools
import sys

for _p in ("/opt/trn_rl_repo",):
    if _p not in sys.path:
        sys.path.insert(0, _p)

import numpy as np

import concourse.mybir as mybir
import concourse.tile as tile
from concourse import bacc
from concourse.bass_utils import run_bass_kernel_spmd

G = 32
IO = 128  # in_o
OO = 128  # out_o
D = G * IO  # 4096
NCORES = 8
B, T = 4, 2048
BT = B * T
TOKC = BT // NCORES  # tokens per core
ST = 512  # supertile tokens
NST = TOKC // ST

DT16 = mybir.dt.float16
DTF = mybir.dt.float32
NP16 = np.float16

TRANSPOSE_MODE = True  # P2/P4 via PE transpose mode -> fp16 PSUM, 2x drains


def _hadamard(n):
    H = np.array([[1.0]], dtype=np.float32)
    while H.shape[0] < n:
        H = np.block([[H, H], [H, -H]])
    return H  # +-1, symmetric


def build_body(nc, tc, xin, hm, idm, wqm, hmb, yout, loop_r=1):
    """Emit the per-core program. All APs are DRAM tensors."""
    CH = ST // 128  # 128-token chunks per supertile

    with (
        tc.tile_pool(name="const", bufs=1) as cpool,
        tc.tile_pool(name="stage", bufs=1) as spool,
        tc.tile_pool(name="xa", bufs=2) as xapool,
        tc.tile_pool(name="yf", bufs=4) as ypool,
        tc.tile_pool(name="psA", bufs=3, space="PSUM") as psA,  # f32 [128,1024]
        tc.tile_pool(name="psB", bufs=2, space="PSUM") as psB,  # f16 [128,1024]
    ):
        hmt = cpool.tile([128, 128], DT16, tag="hm")
        nc.sync.dma_start(hmt[:], hm[:])
        idt = cpool.tile([128, 128], DT16, tag="id")
        nc.sync.dma_start(idt[:], idm[:])
        wqt = cpool.tile([128, G * OO], DT16, tag="wq")
        nc.sync.dma_start(wqt[:], wqm[:])
        hmbt = cpool.tile([128, G * 128], DT16, tag="hmb")
        nc.sync.dma_start(hmbt[:], hmb[:])

        # drain engine split: (engine, n_on_vector out of 16) per pass
        def drain(use_v, dst, src):
            if use_v:
                nc.vector.tensor_copy(dst, src)
            else:
                nc.scalar.copy(dst, src)

        # per-pass count of [128,1024] drains sent to DVE (of 16), applied
        # Bresenham-interleaved; f16 2x drains are cheap on DVE so it takes
        # more of those
        NV_P1, NV_P2, NV_P3, NV_P4, NV_P5 = 7, 10, 7, 10, 7

        def pick_v(nv, idx):
            return ((idx + 1) * nv) // 16 > (idx * nv) // 16

        def supertile(st):
            t0 = st * ST
            # ---- load: ONE straight DMA per supertile; DRAM is pre-tiled so
            # each partition's 32KB is contiguous. xabig col = k*512 + t for
            # feature-block k (feature row = k*128 + partition).
            xabig = xapool.tile([128, 32 * ST], DT16, tag="xab")
            nc.sync.dma_start(xabig[:], xin[st])

            # ---- P1: stationary FHT_g -> token-major tm_c, h-MAJOR columns
            # (col = h*128 + i) so P2's stationary is a contiguous slice and
            # only depends on the kq = h//8 drain of each chunk. kq runs
            # outermost so P2's first needs (kq=0 of all chunks) drain first.
            tms = [spool.tile([128, D], DT16, tag=f"tm{c}", name=f"tm{c}") for c in range(CH)]
            for kq in range(4):
                for c in range(CH):
                    ps = psA.tile([128, 1024], DTF, tag="ps")
                    for kk in range(8):
                        k = kq * 8 + kk
                        nc.tensor.matmul(
                            ps[:, kk * 128 : (kk + 1) * 128],
                            lhsT=xabig[:, k * ST + c * 128 : k * ST + (c + 1) * 128],
                            rhs=hmt[:],
                            start=True,
                            stop=True,
                        )
                    # psum col (kk, i', h) -> tm col h*128 + kq*32 + kk*4 + i'
                    tmv = tms[c].rearrange(
                        "p (h kq kk ip) -> p kq kk ip h", h=32, kq=4, kk=8, ip=4
                    )[:, kq]
                    drain(
                        pick_v(NV_P1, kq * 4 + c),
                        tmv,
                        ps[:].rearrange("p (kk ip h) -> p kk ip h", kk=8, ip=4),
                    )

            # ---- P2: transpose-regroup -> xbbig[:, h*512:(h+1)*512] = [i, tok]
            xbbig = spool.tile([128, G * ST], DT16, tag="xb")
            for h2 in range(16):  # pairs of h
                if TRANSPOSE_MODE:
                    ps = psB.tile([128, 1024], DT16, tag="psb")
                else:
                    ps = psA.tile([128, 1024], DTF, tag="ps")
                for hh in range(2):
                    h = h2 * 2 + hh
                    for c in range(CH):
                        nc.tensor.matmul(
                            ps[:, hh * 512 + c * 128 : hh * 512 + (c + 1) * 128],
                            lhsT=tms[c][:, h * 128 : (h + 1) * 128],
                            rhs=idt[:],
                            start=True,
                            stop=True,
                            is_transpose=TRANSPOSE_MODE or None,
                        )
                drain(pick_v(NV_P2, h2), xbbig[:, h2 * 1024 : (h2 + 1) * 1024], ps[:])

            # ---- P3: stationary grouped matmul -> token-major tm2_c.
            # Drain scatters to o-major column order (col = o*32 + h) so P4's
            # selection is a contiguous 128-column slice (walrus allows
            # only 2D APs on the matmul stationary operand).
            tm2s = []
            for c in range(CH):
                tm2 = spool.tile([128, D], DT16, tag=f"tm2_{c}")
                tm2v = tm2.rearrange("p (o h) -> p h o", h=32)
                for hq in range(4):  # 8 MMs -> one [128,1024] drain
                    ps = psA.tile([128, 1024], DTF, tag="ps")
                    for hh in range(8):
                        h = hq * 8 + hh
                        nc.tensor.matmul(
                            ps[:, hh * 128 : (hh + 1) * 128],
                            lhsT=xbbig[:, h * ST + c * 128 : h * ST + (c + 1) * 128],
                            rhs=wqt[:, h * 128 : (h + 1) * 128],
                            start=True,
                            stop=True,
                        )
                    drain(pick_v(NV_P3, c * 4 + hq), tm2v[:, hq * 8 : (hq + 1) * 8, :], ps[:])
                tm2s.append(tm2)

            # ---- P4: transpose-regroup -> ypa_j (reuses tm buffers; tm is dead
            # after P2). Tile j holds m in [8j, 8j+8): col = (m%8)*512 + t.
            ypas = [
                spool.tile([128, D], DT16, tag=f"tm{j}", name=f"ypa{j}")
                for j in range(4)
            ]
            for m2 in range(16):  # pairs of m
                if TRANSPOSE_MODE:
                    ps = psB.tile([128, 1024], DT16, tag="psb")
                else:
                    ps = psA.tile([128, 1024], DTF, tag="ps")
                for mm in range(2):
                    m = m2 * 2 + mm
                    for c in range(CH):
                        nc.tensor.matmul(
                            ps[:, mm * 512 + c * 128 : mm * 512 + (c + 1) * 128],
                            lhsT=tm2s[c][:, m * 128 : (m + 1) * 128],
                            rhs=idt[:],
                            start=True,
                            stop=True,
                            is_transpose=TRANSPOSE_MODE or None,
                        )
                drain(
                    pick_v(NV_P4, m2),
                    ypas[m2 // 4][:, (m2 % 4) * 1024 : (m2 % 4 + 1) * 1024],
                    ps[:],
                )

            # ---- P5: moving FHT_h with beta folded into per-m stationaries
            for m2 in range(16):
                ps = psA.tile([128, 1024], DTF, tag="ps")
                for mm in range(2):
                    m = m2 * 2 + mm
                    nc.tensor.matmul(
                        ps[:, mm * 512 : (mm + 1) * 512],
                        lhsT=hmbt[:, m * 128 : (m + 1) * 128],
                        rhs=ypas[m // 8][:, (m % 8) * ST : (m % 8 + 1) * ST],
                        start=True,
                        stop=True,
                    )
                yf = ypool.tile([128, 1024], DT16, tag="yf")
                drain(pick_v(NV_P5, m2), yf[:], ps[:])
                # one DMA per m-pair: target rows are contiguous [m2*256, m2*256+256).
                # Issue from Act so stores ride the qActDynamicHW queue while
                # loads ride qSPDynamicHW (each engine has its own HW DGE queue).
                nc.scalar.dma_start(yout[st, m2], yf[:])

        if loop_r == 1:
            for st in range(NST):
                supertile(st)
        else:
            with tc.For_i(0, loop_r, 1):
                for st in range(NST):
                    supertile(st)


@functools.lru_cache(maxsize=4)
def build_program(loop_r=1):
    nc = bacc.Bacc("TRN2", target_bir_lowering=False, debug=False)
    xin = nc.dram_tensor("xin", [NST, 128, 32 * ST], DT16, kind="ExternalInput").ap()
    hm = nc.dram_tensor("hmat", [128, 128], DT16, kind="ExternalInput").ap()
    idm = nc.dram_tensor("ident", [128, 128], DT16, kind="ExternalInput").ap()
    wqm = nc.dram_tensor("wqm", [128, G * OO], DT16, kind="ExternalInput").ap()
    hmb = nc.dram_tensor("hmbeta", [128, G * 128], DT16, kind="ExternalInput").ap()
    yout = nc.dram_tensor("yout", [NST, 16, 128, 2 * ST], DT16, kind="ExternalOutput").ap()
    with tile.TileContext(nc) as tc:
        build_body(nc, tc, xin, hm, idm, wqm, hmb, yout, loop_r=loop_r)
    nc.compile()
    return nc


def host_prep(x, weight, alpha, beta):
    """Returns per-core input maps. Pure f32 numpy glue + fp16 casts."""
    H = _hadamard(G)  # [g,h] +-1

    w = np.asarray(weight, dtype=np.float32)
    scale = np.float32(np.mean(np.abs(w))) + np.float32(1e-8)
    wq3 = np.clip(np.round(w / scale), -1.0, 1.0).astype(np.float32)  # [h,o,i] in {-1,0,1}

    # x * alpha, reorder features to i-major (f' = i*32+g), then transpose to
    # feature-major [D, BT] so device loads are straight contiguous DMA.
    xp = np.asarray(x, dtype=np.float32).reshape(BT, G, IO) * np.asarray(
        alpha, dtype=np.float32
    )[None]
    xp = np.ascontiguousarray(xp.transpose(2, 1, 0)).reshape(D, BT)  # [(i,g), tok]
    xin_all = xp.astype(NP16)

    hmat = np.kron(np.eye(4, dtype=np.float32), H)  # [(i',g),(i'',h)]
    ident = np.eye(128, dtype=np.float32)
    wq_sb = np.ascontiguousarray(wq3.transpose(2, 0, 1)).reshape(IO, G * OO)  # [i,(h,o)]

    # P5 stationaries: hmb[:, m*128+col] = hmat[:, col] * beta_f[g, 4m+o'']
    # where col = o''*32 + g
    beta_f = np.asarray(beta, dtype=np.float32) * (scale / np.float32(G))  # [g,o]
    colscale = np.empty((G, 128), dtype=np.float32)  # [m, col]
    for m in range(G):
        colscale[m] = beta_f[:, 4 * m : 4 * m + 4].T.reshape(128)  # (o'',g)
    hmb = (hmat[:, None, :] * colscale[None, :, :]).reshape(128, G * 128)

    in_maps = []
    for c in range(NCORES):
        in_maps.append(
            {
                "xin": _tile_xin(xin_all[:, c * TOKC : (c + 1) * TOKC]),
                "hmat": hmat.astype(NP16),
                "ident": ident.astype(NP16),
                "wqm": wq_sb.astype(NP16),
                "hmbeta": hmb.astype(NP16),
            }
        )
    return in_maps


def _tile_xin(xc):
    # [D, TOKC] -> [NST, 128, 32*ST]: partition p of supertile st holds
    # feature rows k*128+p for k=0..31, tokens st*ST..+ST, k-major contiguous
    v = xc.reshape(32, 128, NST, ST)  # [k, p, st, t]
    return np.ascontiguousarray(v.transpose(2, 1, 0, 3)).reshape(NST, 128, 32 * ST)


def host_post(results):
    ydev = np.stack([np.asarray(r["yout"], dtype=np.float32) for r in results])
    # ydev [c, st, m2, p, (mm t)]: feature row r = (2*m2+mm)*128 + p,
    # token = st*ST + t; r = m*128 + o'*32 + g -> feature (g, o = 4m+o')
    y = ydev.reshape(NCORES, NST, 16, 128, 2, ST)
    y = y.transpose(0, 2, 4, 3, 1, 5)  # [c, m2, mm, p, st, t]
    y_fm = np.ascontiguousarray(y).reshape(NCORES, D, TOKC)
    y2 = y_fm.reshape(NCORES, G, 4, G, TOKC)  # [c, m, o', g, tok]
    y2 = y2.transpose(0, 4, 3, 1, 2)  # [c, tok, g, m, o']
    y2 = np.ascontiguousarray(y2).reshape(BT, D)
    return y2.reshape(B, T, D)


def kernel(x, weight, alpha, beta):
    nc = build_program(loop_r=1)
    in_maps = host_prep(x, weight, alpha, beta)
    res = run_bass_kernel_spmd(nc, in_maps, core_ids=list(range(NCORES)))
    return host_post(res.results)
